# revision 57
# baseline (speedup 1.0000x reference)
"""DANetHead (position attention + channel attention + conv/BN/ReLU) on 8
Trainium2 NeuronCores via Bass/Tile.

Sharding: data-parallel over batch (4) x image-row-halves (2) = 8 cores.
Each core computes a 34-row window (32 own rows + 1 halo row on each side)
of one batch item, position-uniform across cores via a host-side roll of
the pixel axis; per-core behaviour differs only through input data.

v2 restructure vs the first version:
  - pa is accumulated directly in [c, m] orientation (lhsT = vT c-chunk,
    rhs = expT) so the conv input layout falls out of the bmm with NO PE
    transposes and no separate transposed-residual input; the residual is
    the already-resident xf and the softmax normalization is applied as a
    per-column broadcast (ones-matmul) multiply.
  - q/k are projected with column-duplicated weights to [128, *] so the
    energy matmuls run as two concurrent 64-row tile_position matmuls
    (chunk pair per step) at ~2x effective rate.
  - channel pooling happens right in each block tail (free-axis reduce),
    so the pooled-stats AllGather is issued as soon as block 3 finishes;
    a dummy warmup AllGather at kernel start absorbs collective setup.
  - input DMAs split across the sync and scalar queues.
"""

import numpy as np
import ml_dtypes

import concourse.bass as bass
import concourse.mybir as mybir
import concourse.tile as tile

BF16 = ml_dtypes.bfloat16
F32 = np.float32

P = 128
CIN = 512            # channels
NPIX = 4096          # 64*64 pixels
C8 = 64              # q/k channels
OC = 256             # conv output channels
M = 2176             # per-core pixel window: 34 rows * 64
OWN = 2048           # own pixels: window rows 1..32 -> m 0..2047
NCH = NPIX // P      # 32 n-chunks
NPAIR = NCH // 2     # 16 chunk pairs
REPLICA_GROUPS = [[0, 1], [2, 3], [4, 5], [6, 7]]

BN_EPS = 1e-5

_BUILD_CACHE = {}


def _emit(tc, nc, t):
    fp32 = mybir.dt.float32
    f32r = mybir.dt.float32r
    bf16 = mybir.dt.bfloat16
    Act = mybir.ActivationFunctionType
    Alu = mybir.AluOpType
    AxX = mybir.AxisListType.X

    import contextlib
    ctx = contextlib.ExitStack()

    persist = ctx.enter_context(tc.tile_pool(name="persist", bufs=1))
    vt_pool = ctx.enter_context(tc.tile_pool(name="vt", bufs=NCH))
    expt_pool = ctx.enter_context(tc.tile_pool(name="expt", bufs=16))
    esum_pool = ctx.enter_context(tc.tile_pool(name="esum", bufs=4))
    t1_pool = ctx.enter_context(tc.tile_pool(name="t1", bufs=5))
    recb_pool = ctx.enter_context(tc.tile_pool(name="recb", bufs=2))
    out_pool = ctx.enter_context(tc.tile_pool(name="yout", bufs=3))
    small = ctx.enter_context(tc.tile_pool(name="small", bufs=2))

    ps_e = ctx.enter_context(tc.tile_pool(name="ps_e", bufs=2, space="PSUM"))
    ps_pa = ctx.enter_context(tc.tile_pool(name="ps_pa", bufs=5, space="PSUM"))
    ps_db = ctx.enter_context(tc.tile_pool(name="ps_db", bufs=1, space="PSUM"))

    dram = ctx.enter_context(tc.tile_pool(name="dram", bufs=1, space="DRAM"))

    # ---------------- loads ----------------
    # xf on the sync queue (fine-grained so projections start early);
    # all weights on the gpsimd queue; NOTHING on the scalar queue so
    # the activation stream never stalls behind a DMA issue.
    wq2_sb = persist.tile([P, 4, P], bf16)
    nc.sync.dma_start(out=wq2_sb, in_=t["wq2"].ap())
    wk2_sb = persist.tile([P, 4, P], bf16)
    nc.sync.dma_start(out=wk2_sb, in_=t["wk2"].ap())
    bq2_sb = persist.tile([P, 1], fp32)
    nc.gpsimd.dma_start(out=bq2_sb, in_=t["bq2"][:, :])
    bk2_sb = persist.tile([P, 1], fp32)
    nc.gpsimd.dma_start(out=bk2_sb, in_=t["bk2"][:, :])

    # wvT + half the xf stream ride the scalar queue (it is idle until the
    # first projection activation at ~13us); the rest goes on sync.  Two
    # queues double the DMA issue/credit throughput for the 4MB x load.
    wvT_sb = persist.tile([P, 4, CIN], bf16)
    nc.scalar.dma_start(out=wvT_sb, in_=t["wvT"].ap())

    xf_sb = [persist.tile([P, NPIX], bf16, name=f"xf{ci}") for ci in range(4)]
    for sl in (slice(0, 1024), slice(1024, 2048),
               slice(2048, 3072), slice(3072, 4096)):
        for ci in range(4):
            eng = nc.sync if ci < 2 else nc.scalar
            eng.dma_start(out=xf_sb[ci][:, sl],
                          in_=t["xf"][ci * P:(ci + 1) * P, sl])

    # warmup collective (absorb CC setup latency; result unused)
    wi_sb = small.tile([1, 8], fp32, tag="wi", bufs=1)
    nc.vector.memset(wi_sb, 0.0)
    warm_in = dram.tile([1, 8], fp32, tag="warm_in")
    warm_out = dram.tile([2, 8], fp32, tag="warm_out")
    nc.sync.dma_start(out=warm_in, in_=wi_sb)
    nc.gpsimd.collective_compute("AllGather", Alu.bypass,
                                 replica_groups=REPLICA_GROUPS,
                                 ins=[warm_in.opt()], outs=[warm_out.opt()])

    # ---------------- q / k projections (row-duplicated to 128) --------
    # qT columns (m): own rows 1..32 -> xf cols 64..2112, then halo rows
    # 0, 33 -> xf cols 0..64 and 2112..2176.
    qT_sb = persist.tile([P, M], bf16)
    k_sb = persist.tile([P, NPIX], bf16)

    def emit_k(off):
        k_ps = ps_e.tile([P, 512], fp32, tag="e")
        for ci in range(4):
            nc.tensor.matmul(k_ps, lhsT=wk2_sb[:, ci, :],
                             rhs=xf_sb[ci][:, off:off + 512],
                             start=(ci == 0), stop=(ci == 3))
        nc.scalar.activation(k_sb[:, off:off + 512], k_ps,
                             Act.Identity, bias=bk2_sb[:, 0:1])

    def emit_q(off):
        q_ps = ps_e.tile([P, 512], fp32, tag="e")
        for ci in range(4):
            nc.tensor.matmul(q_ps, lhsT=wq2_sb[:, ci, :],
                             rhs=xf_sb[ci][:, 64 + off:64 + off + 512],
                             start=(ci == 0), stop=(ci == 3))
        nc.scalar.activation(qT_sb[:, off:off + 512], q_ps,
                             Act.Identity, bias=bq2_sb[:, 0:1])

    # interleaved to match the xf piece arrival order
    emit_k(0)
    emit_q(0)
    emit_k(512)
    emit_q(512)
    emit_k(1024)
    emit_q(1024)
    emit_k(1536)
    emit_q(1536)

    # ---------------- v^T ----------------
    def emit_vt(nch):
        v_ps = ps_e.tile([P, 512], fp32, tag="e")
        for ci in range(4):
            nc.tensor.matmul(v_ps,
                             lhsT=xf_sb[ci][:, nch * P:(nch + 1) * P],
                             rhs=wvT_sb[:, ci, :],
                             start=(ci == 0), stop=(ci == 3))
        vt = vt_pool.tile([P, CIN], bf16, tag="vt", name=f"vt{nch}")
        nc.vector.tensor_copy(vt, v_ps)
        vt_sb.append(vt)

    # first-half v chunks before any second-half consumer so the PE never
    # head-of-line blocks on the late xf pieces
    vt_sb = []
    for nch in range(16):
        emit_vt(nch)

    qh_ps = ps_e.tile([P, P], fp32, tag="e")
    for ci in range(4):
        nc.tensor.matmul(qh_ps[:, 0:64], lhsT=wq2_sb[:, ci, :],
                         rhs=xf_sb[ci][:, 0:64],
                         start=(ci == 0), stop=(ci == 3))
    for ci in range(4):
        nc.tensor.matmul(qh_ps[:, 64:128], lhsT=wq2_sb[:, ci, :],
                         rhs=xf_sb[ci][:, OWN + 64:OWN + 128],
                         start=(ci == 0), stop=(ci == 3))
    nc.scalar.activation(qT_sb[:, OWN:OWN + P], qh_ps,
                         Act.Identity, bias=bq2_sb[:, 0:1])
    for off in range(2048, NPIX, 512):       # k second half
        emit_k(off)
    for nch in range(16, NCH):
        emit_vt(nch)

    # ---- late loads (gpsimd queue; not needed until tails / conv).
    # All tensors are host-prearranged to the SBUF layout so the DMAs are
    # contiguous per partition.
    xres_sb = persist.tile([P, 4, P], bf16)
    nc.gpsimd.dma_start(out=xres_sb, in_=t["xres"].ap())
    pmask_sb = persist.tile([1, P], bf16)
    nc.gpsimd.dma_start(out=pmask_sb, in_=t["pmask"][:, :])
    bvp_sb = persist.tile([P, 4], fp32)
    nc.gpsimd.dma_start(out=bvp_sb, in_=t["bvp"].ap())
    w1T_sb = persist.tile([P, 4, C8], bf16)
    nc.gpsimd.dma_start(out=w1T_sb, in_=t["w1T"].ap())
    w2T_sb = persist.tile([C8, 4, P], bf16)
    nc.gpsimd.dma_start(out=w2T_sb,
                        in_=t["w2T"].ap().rearrange("k (c p) -> k c p", p=P))
    cw_sb = persist.tile([P, 36, OC], bf16)
    nc.gpsimd.dma_start(out=cw_sb, in_=t["cw"].ap())
    bns_sb = persist.tile([P, 2], fp32)
    nc.gpsimd.dma_start(out=bns_sb, in_=t["bns"].ap())
    bnb_sb = persist.tile([P, 2], fp32)
    nc.gpsimd.dma_start(out=bnb_sb, in_=t["bnb"].ap())

    # ---------------- position attention ----------------
    # ca: [c-part, 4 c-groups, 34 rows x 66 cols], zero col pads.
    ca_sb = persist.tile([P, 4, 34 * 66], bf16)
    for cc in range(4):
        cav = ca_sb[:, cc, :].rearrange("p (r x) -> p r x", x=66)
        nc.vector.memset(cav[:, :, 0:1], 0.0)
        nc.vector.memset(cav[:, :, 65:66], 0.0)

    pool_s = small.tile([P, 4], fp32, tag="pool_s", bufs=1)
    pool_m = small.tile([P, 4], fp32, tag="pool_m", bufs=1)
    pool_sm = small.tile([P, P], bf16, tag="pool_sm", bufs=1)
    nc.vector.memset(pool_sm, 0.0)

    ones_col = small.tile([P, 1], bf16, tag="ones_c", bufs=1)
    nc.vector.memset(ones_col, 1.0)
    ones_row = small.tile([1, P], bf16, tag="ones_r", bufs=1)
    nc.vector.memset(ones_row, 1.0)

    # pad-column mask broadcast to [P, P] once (off the critical path)
    pmb_ps = ps_db.tile([P, P], fp32, tag="db")
    nc.tensor.matmul(pmb_ps, lhsT=ones_row, rhs=pmask_sb,
                     start=True, stop=True)
    pmask_bc = small.tile([P, P], bf16, tag="pmask_bc", bufs=1)
    nc.vector.tensor_copy(pmask_bc, pmb_ps)

    def emit_block(bi, boff, bsz, halo):
        """One m-block: energy pairs + exp + pa accumulation + esum.

        The two esum accumulators run on different engines (vector and
        gpsimd) so the serial accumulation chains are half-length and the
        expt ring never waits on a vector-queue backlog.
        """
        pa_ps = [ps_pa.tile([P, bsz], fp32, tag="pa_acc", name=f"pa{bi}_{cc}")
                 for cc in range(4)]
        esum_a = esum_pool.tile([P, bsz], bf16, tag="esum", name=f"esa{bi}")
        esum_b = esum_pool.tile([P, bsz], bf16, tag="esum", name=f"esb{bi}")
        # Software-pipelined: the PE queue is in-order, so e-pair(t+1) is
        # emitted BEFORE pa(t) — the PE streams e(t+1) while the scalar
        # engine computes exp(t), and pa(t) follows with exp(t) long done.
        prev = None
        for tp in range(NPAIR + 1):
            cur = None
            if tp < NPAIR:
                e_a = ps_e.tile([P, bsz], fp32, tag="e")
                e_b = ps_e.tile([P, bsz], fp32, tag="e")
                n0, n1 = 2 * tp, 2 * tp + 1
                nc.tensor.matmul(e_a, lhsT=k_sb[0:64, n0 * P:(n0 + 1) * P],
                                 rhs=qT_sb[0:64, boff:boff + bsz],
                                 start=True, stop=True)
                nc.tensor.matmul(e_b, lhsT=k_sb[64:128, n1 * P:(n1 + 1) * P],
                                 rhs=qT_sb[64:128, boff:boff + bsz],
                                 start=True, stop=True)
                expt_a = expt_pool.tile([P, bsz], bf16, tag="expt")
                expt_b = expt_pool.tile([P, bsz], bf16, tag="expt")
                nc.scalar.activation(expt_a, e_a, Act.Exp)
                nc.scalar.activation(expt_b, e_b, Act.Exp)
                cur = (expt_a, expt_b, n0, n1, tp)
            if prev is not None:
                pexpt_a, pexpt_b, pn0, pn1, ptp = prev
                for cc in range(4):
                    nc.tensor.matmul(pa_ps[cc],
                                     lhsT=vt_sb[pn0][:, cc * P:(cc + 1) * P],
                                     rhs=pexpt_a, start=(ptp == 0), stop=False)
                for cc in range(4):
                    nc.tensor.matmul(pa_ps[cc],
                                     lhsT=vt_sb[pn1][:, cc * P:(cc + 1) * P],
                                     rhs=pexpt_b, start=False,
                                     stop=(ptp == NPAIR - 1))
                with nc.allow_low_precision(reason="softmax denoms, bf16"):
                    if ptp == 0:
                        nc.vector.tensor_copy(esum_a, pexpt_a)
                        nc.vector.tensor_copy(esum_b, pexpt_b)
                    else:
                        nc.vector.tensor_add(esum_a, esum_a, pexpt_a)
                        nc.vector.tensor_add(esum_b, esum_b, pexpt_b)
            prev = cur
        return pa_ps, (esum_a, esum_b)

    def block_tail(bi, boff, bsz, pa_ps, esums, halo):
        """normalize (per-column), residual, pooling, ca write.

        Work is spread over vector (normalize+residual), gpsimd (pooling)
        and scalar (ca writes) so no single queue serializes the tail.
        """
        esum_a, esum_b = esums
        # copy pa out of PSUM right away (split DVE/ACT) so the next
        # block's accumulation never waits on the denominator chain
        t1s = []
        for cc in range(4):
            t1 = t1_pool.tile([P, bsz], bf16, tag="t1")
            if cc % 2 == 0:
                nc.vector.tensor_copy(t1, pa_ps[cc])
            else:
                nc.scalar.activation(t1, pa_ps[cc], Act.Identity)
            t1s.append(t1)
        den_ps = ps_db.tile([1, bsz], fp32, tag="db")
        nc.tensor.matmul(den_ps, lhsT=ones_col, rhs=esum_a,
                         start=True, stop=False)
        nc.tensor.matmul(den_ps, lhsT=ones_col, rhs=esum_b,
                         start=False, stop=True)
        den_row = small.tile([1, bsz], bf16, tag="den_row")
        nc.vector.tensor_copy(den_row, den_ps)
        recb_ps = ps_db.tile([P, bsz], fp32, tag="db")
        nc.tensor.matmul(recb_ps, lhsT=ones_row, rhs=den_row,
                         start=True, stop=True)
        recb = recb_pool.tile([P, bsz], fp32, tag="recb")
        nc.vector.reciprocal(recb, recb_ps)
        if halo:
            nc.vector.tensor_mul(recb, recb, pmask_bc)
        for cc in range(4):
            nc.vector.tensor_mul(t1s[cc], t1s[cc], recb)
        cav = ca_sb.rearrange("p c (r x) -> p c r x", x=66)
        if not halo:
            ptmp = small.tile([P, 4, 2], fp32, tag="ptmp")
            for cc in range(4):
                nc.vector.tensor_add(t1s[cc], t1s[cc],
                                     xf_sb[cc][:, 64 + boff:64 + boff + bsz])
            for cc in range(4):
                t1 = t1s[cc]
                if bi == 0:
                    nc.vector.reduce_sum(pool_s[:, cc:cc + 1], t1, axis=AxX)
                    nc.vector.reduce_max(pool_m[:, cc:cc + 1], t1, axis=AxX)
                else:
                    nc.vector.reduce_sum(ptmp[:, cc, 0:1], t1, axis=AxX)
                    nc.vector.reduce_max(ptmp[:, cc, 1:2], t1, axis=AxX)
                    nc.vector.tensor_add(pool_s[:, cc:cc + 1],
                                         pool_s[:, cc:cc + 1], ptmp[:, cc, 0:1])
                    nc.vector.tensor_max(pool_m[:, cc:cc + 1],
                                         pool_m[:, cc:cc + 1], ptmp[:, cc, 1:2])

            def ca_writes():
                for cc in range(4):
                    r0 = 1 + 8 * bi
                    nc.vector.tensor_scalar(
                        out=cav[:, cc, r0:r0 + 8, 1:65],
                        in0=t1s[cc].rearrange("p (r x) -> p r x", x=64),
                        scalar1=bvp_sb[:, cc:cc + 1], scalar2=None,
                        op0=Alu.add)
        else:
            for cc in range(4):
                nc.vector.tensor_add(t1s[cc], t1s[cc], xres_sb[:, cc, :])

            def ca_writes():
                for cc in range(4):
                    t1 = t1s[cc]
                    nc.vector.tensor_copy(
                        cav[:, cc, 0:1, 1:65],
                        t1[:, 0:64].rearrange("p (r x) -> p r x", x=64))
                    nc.vector.tensor_copy(
                        cav[:, cc, 33:34, 1:65],
                        t1[:, 64:128].rearrange("p (r x) -> p r x", x=64))
        return ca_writes

    # blocks 0..3: own rows.  Block 3's ca writes are deferred until the
    # pooled-stats AllGather has been issued (they are not on its path).
    for bi in range(4):
        pa_ps, esum = emit_block(bi, 512 * bi, 512, halo=False)
        ca_w = block_tail(bi, 512 * bi, 512, pa_ps, esum, halo=False)
        if bi < 3:
            ca_w()

    # ------- pooled-stats AllGather (issued right after block 3) -------
    # Transposed to [8, P] rows so the DRAM DMAs are contiguous packets
    # instead of a 128-partition scatter.
    ag_in = dram.tile([8, P], bf16, tag="ag_in")
    ag_out = dram.tile([2, 8, P], bf16, tag="ag_out")
    nc.vector.tensor_copy(pool_sm[:, 0:4], pool_s)
    nc.vector.tensor_copy(pool_sm[:, 4:8], pool_m)
    pool_smT = small.tile([P, P], bf16, tag="pool_smT", bufs=1)
    nc.sync.dma_start(out=pool_smT, in_=pool_sm, transpose=True)
    nc.sync.dma_start(out=ag_in, in_=pool_smT[0:8, :])
    nc.gpsimd.collective_compute("AllGather", Alu.bypass,
                                 replica_groups=REPLICA_GROUPS,
                                 ins=[ag_in.opt()], outs=[ag_out.opt()])

    # halo block (rows 0 and 33) overlaps the collective; block 3's ca
    # writes go after the halo emission so the halo's exp stream is not
    # serialized behind them in the queues.
    pa_ps, esum = emit_block(4, OWN, P, halo=True)
    ca_w()
    ca_wh = block_tail(4, OWN, P, pa_ps, esum, halo=True)
    ca_wh()

    agT = small.tile([16, P], bf16, tag="agT", bufs=1)
    nc.sync.dma_start(out=agT, in_=ag_out.rearrange("m e p -> (m e) p"))
    zall = small.tile([P, 16], bf16, tag="zall", bufs=1)
    nc.sync.dma_start(out=zall, in_=agT, transpose=True)
    zv = zall.rearrange("p (m s c) -> p m s c", m=2, s=2)
    zs_sb = small.tile([P, 4], fp32, tag="zs")
    zm_sb = small.tile([P, 4], fp32, tag="zm")
    nc.vector.tensor_add(zs_sb, zv[:, 0, 0, :], zv[:, 1, 0, :])
    nc.vector.tensor_max(zm_sb, zv[:, 0, 1, :], zv[:, 1, 1, :])

    # ---------------- SE MLP + sigmoid ----------------
    # pooled stats exclude bv (folded residual misses it): fix up here.
    rhs_z = small.tile([P, 4, 2], bf16, tag="rhs_z")
    zt = small.tile([P, 4], fp32, tag="zt")
    nc.vector.tensor_scalar_mul(zt, zs_sb, 1.0 / float(NPIX))
    nc.vector.tensor_add(rhs_z[:, :, 0], zt, bvp_sb)
    nc.vector.tensor_add(rhs_z[:, :, 1], zm_sb, bvp_sb)

    h_ps = ps_db.tile([C8, 2], fp32, tag="db")
    for cc in range(4):
        nc.tensor.matmul(h_ps, lhsT=w1T_sb[:, cc, :], rhs=rhs_z[:, cc, :],
                         start=(cc == 0), stop=(cc == 3))
    h_sb = small.tile([C8, 2], bf16, tag="h_sb")
    nc.vector.tensor_scalar_max(h_sb, h_ps, 0.0)

    stot = small.tile([P, 4], fp32, tag="stot")
    s_sb = small.tile([P, 4, 2], fp32, tag="s_sb", bufs=1)
    for cc in range(4):
        s_ps = ps_pa.tile([P, 2], fp32, tag="pa_acc")
        nc.tensor.matmul(s_ps, lhsT=w2T_sb[:, cc, :], rhs=h_sb,
                         start=True, stop=True)
        if cc % 2 == 0:
            nc.vector.tensor_copy(s_sb[:, cc, :], s_ps)
        else:
            nc.scalar.activation(s_sb[:, cc, :], s_ps, Act.Identity)
    nc.vector.tensor_add(stot, s_sb[:, :, 0], s_sb[:, :, 1])

    es = small.tile([P, 4], fp32, tag="es")
    nc.scalar.activation(es, stot, Act.Exp, scale=-1.0)
    nc.vector.tensor_scalar_add(es, es, 1.0)
    scale_sb = small.tile([P, 4], fp32, tag="scale")
    nc.vector.reciprocal(scale_sb, es)

    # fold the per-input-channel SE scale into the conv weights
    # (split across the vector and scalar engines)
    cwS = persist.tile([P, 36, OC], bf16)
    cwv_in = cw_sb.rearrange("p (t c) o -> p c t o", c=4)
    cwv_out = cwS.rearrange("p (t c) o -> p c t o", c=4)
    for cc in range(4):
        if cc < 2:
            nc.vector.tensor_scalar(out=cwv_out[:, cc], in0=cwv_in[:, cc],
                                    scalar1=scale_sb[:, cc:cc + 1],
                                    scalar2=None, op0=Alu.mult)
        else:
            nc.scalar.activation(cwv_out[:, cc], cwv_in[:, cc],
                                 Act.Identity, scale=scale_sb[:, cc:cc + 1])

    # ---------------- conv 3x3 + BN + ReLU ----------------
    for pt in range(4):
        for oc in range(2):
            y_ps = ps_pa.tile([P, 512], fp32, tag="pa_acc")
            idx = 0
            for kh in range(3):
                for kw in range(3):
                    tnum = 3 * kh + kw
                    rs = 1 + 8 * pt + (kh - 1)
                    for ci in range(4):
                        rhs = (ca_sb[:, ci, :]
                               .rearrange("p (r x) -> p r x", x=66)
                               [:, rs:rs + 8, kw:kw + 64])
                        nc.tensor.matmul(
                            y_ps, lhsT=cwS[:, tnum * 4 + ci,
                                           oc * P:(oc + 1) * P],
                            rhs=rhs, start=(idx == 0), stop=(idx == 35))
                        idx += 1
            y_sb = out_pool.tile([P, 512], fp32, tag="y_sb")
            nc.scalar.activation(y_sb, y_ps, Act.Relu,
                                 bias=bnb_sb[:, oc:oc + 1],
                                 scale=bns_sb[:, oc:oc + 1])
            nc.sync.dma_start(
                out=t["out"][oc * P:(oc + 1) * P, pt * 512:(pt + 1) * 512],
                in_=y_sb)

    ctx.close()


def build():
    """Build (and cache) the SPMD Bass program."""
    if "nc" in _BUILD_CACHE:
        return _BUILD_CACHE["nc"]
    from concourse import bacc
    nc = bacc.Bacc("TRN2", target_bir_lowering=False, num_devices=8)
    f32 = mybir.dt.float32
    bf16 = mybir.dt.bfloat16
    t = {}
    t["xf"] = nc.dram_tensor("xf", [CIN, NPIX], bf16, kind="ExternalInput")
    t["xres"] = nc.dram_tensor("xres", [P, 4, P], bf16, kind="ExternalInput")
    t["pmask"] = nc.dram_tensor("pmask", [1, P], bf16, kind="ExternalInput")
    t["wq2"] = nc.dram_tensor("wq2", [P, 4, P], bf16, kind="ExternalInput")
    t["wk2"] = nc.dram_tensor("wk2", [P, 4, P], bf16, kind="ExternalInput")
    t["bq2"] = nc.dram_tensor("bq2", [P, 1], f32, kind="ExternalInput")
    t["bk2"] = nc.dram_tensor("bk2", [P, 1], f32, kind="ExternalInput")
    t["wvT"] = nc.dram_tensor("wvT", [P, 4, CIN], bf16, kind="ExternalInput")
    t["bvp"] = nc.dram_tensor("bvp", [P, 4], f32, kind="ExternalInput")
    t["w1T"] = nc.dram_tensor("w1T", [P, 4, C8], bf16, kind="ExternalInput")
    t["w2T"] = nc.dram_tensor("w2T", [C8, CIN], bf16, kind="ExternalInput")
    t["cw"] = nc.dram_tensor("cw", [P, 36, OC], bf16, kind="ExternalInput")
    t["bns"] = nc.dram_tensor("bns", [P, 2], f32, kind="ExternalInput")
    t["bnb"] = nc.dram_tensor("bnb", [P, 2], f32, kind="ExternalInput")
    t["out"] = nc.dram_tensor("out", [OC, OWN], f32, kind="ExternalOutput")

    with tile.TileContext(nc) as tc:
        _emit(tc, nc, t)
    nc.compile()

    _BUILD_CACHE["nc"] = nc
    return nc


def make_in_maps(x, wq, bq, wk, bk, wv, bv, ca_w1, ca_w2, conv_w,
                 bn_gamma, bn_beta, bn_mean, bn_var):
    x = np.ascontiguousarray(np.asarray(x, F32))
    B = x.shape[0]
    xf_full = x.reshape(B, CIN, NPIX)

    def part4(a):
        """[4*P, X...] -> [P, 4, X...] (channel c = cc*128 + p)."""
        return np.ascontiguousarray(
            a.reshape(4, P, *a.shape[1:]).transpose(
                1, 0, *range(2, a.ndim + 1)))

    wqT = np.asarray(wq, F32).T          # [CIN, C8]
    wkT = np.asarray(wk, F32).T
    # cw9: [9, CIN, OC] tap-major conv weights
    cw9 = np.stack([np.asarray(conv_w, F32)[:, :, kh, kw].T
                    for kh in range(3) for kw in range(3)])
    # -> [P, (t c), OC]
    cwp = np.ascontiguousarray(
        cw9.reshape(9, 4, P, OC).transpose(2, 0, 1, 3).reshape(P, 36, OC))
    common = {
        "wq2": part4(np.concatenate([wqT, wqT], axis=1)).astype(BF16),
        "wk2": part4(np.concatenate([wkT, wkT], axis=1)).astype(BF16),
        "bq2": np.tile(np.asarray(bq, F32).reshape(C8, 1), (2, 1)),
        "bk2": np.tile(np.asarray(bk, F32).reshape(C8, 1), (2, 1)),
        "wvT": part4(np.asarray(wv, F32).T).astype(BF16),
        "bvp": np.ascontiguousarray(
            np.asarray(bv, F32).reshape(4, P).T),
        "w1T": part4(np.asarray(ca_w1, F32).T).astype(BF16),
        "w2T": np.ascontiguousarray(np.asarray(ca_w2, F32).T.astype(BF16)),
        "cw": cwp.astype(BF16),
    }
    bns = (np.asarray(bn_gamma, F32)
           / np.sqrt(np.asarray(bn_var, F32) + BN_EPS)).astype(F32)
    bnb = (np.asarray(bn_beta, F32) - np.asarray(bn_mean, F32) * bns).astype(F32)
    common["bns"] = np.ascontiguousarray(bns.reshape(2, P).T)
    common["bnb"] = np.ascontiguousarray(bnb.reshape(2, P).T)

    bv_f = np.asarray(bv, F32)
    in_maps = []
    for core in range(8):
        b, h = core // 2, core % 2
        r0 = 32 * h - 1                       # first window row (may be -1)
        rolled = np.roll(xf_full[b], -r0 * 64, axis=1)
        # halo residual (+bv), zeroed on the pad row
        xres = np.empty((CIN, P), F32)
        xres[:, 0:64] = rolled[:, 0:64] + bv_f[:, None]      # window row 0
        xres[:, 64:128] = rolled[:, 2112:2176] + bv_f[:, None]  # row 33
        pmask = np.ones((1, P), F32)
        if h == 0:
            xres[:, 0:64] = 0.0
            pmask[0, 0:64] = 0.0
        else:
            xres[:, 64:128] = 0.0
            pmask[0, 64:128] = 0.0
        in_maps.append(dict(
            common,
            xf=np.ascontiguousarray(rolled.astype(BF16)),
            xres=part4(xres).astype(BF16),
            pmask=pmask.astype(BF16),
        ))
    return in_maps


def assemble(results):
    out = np.zeros((4, OC, 64, 64), F32)
    for core in range(8):
        b, h = core // 2, core % 2
        out[b, :, 32 * h:32 * h + 32, :] = \
            results[core]["out"].reshape(OC, 32, 64)
    return out


def kernel(**inputs):
    from concourse.bass_utils import run_bass_kernel_spmd
    nc = build()
    in_maps = make_in_maps(**inputs)
    res = run_bass_kernel_spmd(nc, in_maps, core_ids=list(range(8)))
    return assemble(res.results)
